# revision 50
# baseline (speedup 1.0000x reference)
"""BERT self-attention (B=4, S=2048, E=768, H=12) on 8 TRN2 NeuronCores.

Sharding: (batch, head-half) — core c handles batch c//2, heads 6*(c%2)..+6.
Each core is fully independent (no collectives).

Host-side prep (in kernel()): per-core shard slicing plus layout/precision
prep — hidden/W transposed to put the contraction dim on partitions, Wq/bq
pre-scaled by 1/sqrt(D), attention_mask folded into domain mask and the
combined mask EXPONENTIATED on the host (E_T = exp(maskT) ships as bf16, so
ScalarE never touches the masks), matmul operands fed as bf16.

Device-side structure (per core):
  - projections (bf16): qT,kT in [o,m] layout; v in [m,o] layout augmented
    with a ones column per head (softmax denominators via the PV matmul).
  - scores^T[k,q] = kT.T @ qT, two heads row-packed per PE pass (d=64 each)
    into one f32 PSUM tile [128, 1024].
  - one ACT pass per k-chunk: exp(scores) PSUM -> SBUF bf16 (the ScalarE
    bottleneck, ~1.0 us per 128x1024 tile).
  - host-precomputed E_T = exp(maskT) multiplied in at bf16 2x on DVE:
    prod = exp_s * E_T.
  - PV: ctx_u^T[65,q] = v_aug.T @ prod accumulated over 16 k-chunks in
    PSUM; row 64 is the softmax denominator.
  - ctx_u^T is copied f32 PSUM->SBUF and DMA'd out UNNORMALIZED; the host
    divides rows 0..63 by row 64 and transposes to [q, e]. This removes
    the per-block PE transposes + DVE normalize from the device entirely.

Pipelining: one global software pipeline over all 192 (q-block, head-pair,
k-chunk) tiles — the next tile's QK matmuls are always emitted before the
previous tile's exp/mult/PV tail, so neither PE nor ScalarE stalls at block
boundaries. Projections are interleaved as filler into the PE slack of the
ACT-bound k-loop with just-in-time deadlines.

Measured on 8 axon TRN2 cores: see test.py output.
"""

import sys

if "/opt/trn_rl_repo" not in sys.path:
    sys.path.insert(0, "/opt/trn_rl_repo")

from contextlib import ExitStack

import ml_dtypes
import numpy as np

import concourse.bass as bass
import concourse.tile as tile
from concourse import bacc, mybir
from concourse.bass_utils import run_bass_kernel_spmd

B, S, E, H = 4, 2048, 768, 12
D = 64
N_CORES = 8
HPC = 6            # heads per core
EC = HPC * D       # 384 embedding cols per core
NIC = E // 128     # 6 contraction chunks
NOC = EC // 128    # 3 output chunks (= head pairs)
NKC = S // 128     # 16 k chunks
QW = 512           # q tile width
NQQ = S // QW      # 4 q chunks
NT = NOC * NQQ * NKC  # 192 tiles total

F32 = mybir.dt.float32
BF16 = mybir.dt.bfloat16
FP8 = mybir.dt.float8e4
DR = mybir.MatmulPerfMode.DoubleRow
Exp = mybir.ActivationFunctionType.Exp


def _emit(ctx: ExitStack, tc: tile.TileContext, h):
    nc = tc.nc

    persist = ctx.enter_context(tc.tile_pool(name="persist", bufs=1))
    consts = ctx.enter_context(tc.tile_pool(name="consts", bufs=1))

    bq_sb = consts.tile([128, NOC], F32)
    bk_sb = consts.tile([128, NOC], F32)
    ones1 = consts.tile([1, 128], BF16)
    scratch1 = consts.tile([1, 1], BF16)

    # ---- persistent activations ----
    qT = persist.tile([128, NOC, S], BF16)        # [o%128, o-chunk, m]
    kT = persist.tile([128, NOC, S], BF16)
    vaug = persist.tile([128, NKC, HPC, D + 4], BF16)  # [m%128, m-chunk, head, d|one]
    ET = persist.tile([128, NKC, S], BF16)        # host exp(maskT), [k%128, k-chunk, q]

    # stage A/B inputs stay resident the whole run (projections interleave
    # into the attention loop)
    sab = ctx.enter_context(tc.tile_pool(name="stageAB", bufs=1))
    xTb = sab.tile([128, NIC, S], BF16)
    wqb = sab.tile([128, NIC, EC], BF16)
    wkb = sab.tile([128, NIC, EC], BF16)
    wvb = sab.tile([128, NIC, EC], BF16)

    # ---- input DMAs: FIRST emissions, spread across three trigger queues
    # so the ramp-critical set (wq/wk + xT mq0) is all in flight within a
    # few triggers of kernel start. ScalarE is idle until the first exp
    # (~17us), so it can serve as a trigger queue for the wk loads.
    def load_x(mq, c):
        qs = slice(mq * QW, (mq + 1) * QW)
        nc.sync.dma_start(
            out=xTb[:, 2 * c : 2 * c + 2, qs],
            in_=h["xT"].ap()[c * 256 : (c + 1) * 256, qs].rearrange(
                "(a p) q -> p a q", p=128
            ),
        )

    def load_w(eng, name, wtb, c):
        eng.dma_start(
            out=wtb[:, 2 * c : 2 * c + 2, :],
            in_=h[name].ap()[c * 256 : (c + 1) * 256, :].rearrange(
                "(a p) o -> p a o", p=128
            ),
        )

    def load_et(eng, kc, qq):
        qs = slice(qq * QW, (qq + 1) * QW)
        eng.dma_start(
            out=ET[:, kc, qs], in_=h["emaskT"].ap()[kc * 128 : (kc + 1) * 128, qs]
        )

    # E_T streams by q-quarter in block consumption order: block (j0,qq0)
    # only reads q-columns 0:512 of every k-chunk, so the first 2.1MB of
    # mask unblocks the whole first block instead of the full 8.4MB.
    for c in range(NIC // 2):
        load_x(0, c)
        load_w(nc.scalar, "wkT", wkb, c)
        load_w(nc.gpsimd, "wqT", wqb, c)
    for kc in range(4):
        load_et(nc.sync, kc, 0)
    for c in range(NIC // 2):
        load_x(1, c)
        load_w(nc.gpsimd, "wvT", wvb, c)
    nc.gpsimd.dma_start(out=bq_sb[:], in_=h["bq"].ap())
    nc.gpsimd.dma_start(out=bk_sb[:], in_=h["bk"].ap())
    for kc in range(4, 10):
        load_et(nc.sync, kc, 0)
    for c in range(NIC // 2):
        load_x(2, c)
    for kc in range(10, NKC):
        load_et(nc.sync, kc, 0)
    for c in range(NIC // 2):
        load_x(3, c)
    for qq in range(1, NQQ):
        for kc in range(NKC):
            load_et(nc.gpsimd, kc, qq)

    # ---- constants (after the DMA triggers so they don't delay them) ----
    nc.vector.memset(ones1[:], 1.0)
    nc.vector.memset(vaug[:, :, :, D : D + 1], 1.0)
    # dummy exp at t~0: pulls the ACT exp-table load off the critical path
    nc.scalar.activation(scratch1[:], ones1[0:1, 0:1], Exp)

    # ---- working pools ----
    sps = ctx.enter_context(tc.tile_pool(name="s_psum", bufs=2, space="PSUM"))
    cps = ctx.enter_context(tc.tile_pool(name="ctx_psum", bufs=1, space="PSUM"))
    pps = ctx.enter_context(tc.tile_pool(name="proj_psum", bufs=2, space="PSUM"))
    expool = ctx.enter_context(tc.tile_pool(name="expool", bufs=10))
    prpool = ctx.enter_context(tc.tile_pool(name="prpool", bufs=6))
    owork = ctx.enter_context(tc.tile_pool(name="owork", bufs=3))

    # ---- projection units, split into single-matmul thunks so they can be
    # interleaved finely into the PE slack of the ACT-bound k-loop ----
    def proj_qk_thunks(dst, wtb, bias, oc, mq):
        st = {}

        def mm(ic):
            def f():
                if ic == 0:
                    st["ps"] = pps.tile([128, QW], F32, tag="pp", name="projps")
                nc.tensor.matmul(
                    st["ps"][:],
                    wtb[:, ic, oc * 128 : (oc + 1) * 128],
                    xTb[:, ic, mq * QW : (mq + 1) * QW],
                    start=(ic == 0),
                    stop=(ic == NIC - 1),
                )
            return f

        def fin():
            nc.vector.tensor_scalar_add(
                dst[:, oc, mq * QW : (mq + 1) * QW], st["ps"][:], bias[:, oc : oc + 1]
            )

        return [("pe", mm(ic)) for ic in range(NIC)] + [("dve", fin)]

    def proj_v_thunks(mc):
        st = {}

        def mm(ic):
            def f():
                if ic == 0:
                    st["ps"] = pps.tile([128, QW], F32, tag="pp", name="projps")
                nc.tensor.matmul(
                    st["ps"][:, 0:EC],
                    xTb[:, ic, mc * 128 : (mc + 1) * 128],
                    wvb[:, ic, :],
                    start=(ic == 0),
                    stop=(ic == NIC - 1),
                )
            return f

        def fin():
            nc.vector.tensor_copy(
                vaug[:, mc, :, 0:D], st["ps"][:, 0:EC].rearrange("p (h d) -> p h d", h=HPC)
            )

        return [("pe", mm(ic)) for ic in range(NIC)] + [("dve", fin)]

    # (deadline_tile, thunks) in consumption order:
    #   v(mc) consumed by the PV of tile mc, which pops ~5 emissions behind
    #   (split mult/PV streams) -> deadline mc+5; kT(0,m) at 4m (QK side);
    #   qT(0,qq) at 16qq; kT(j,m) at 64j; qT(j,qq) at 64j+16qq.
    units = []
    for mc in range(2, NKC):
        units.append((mc + 5, proj_v_thunks(mc)))
    for m in range(1, NQQ):
        units.append((4 * m, proj_qk_thunks(kT, wkb, bk_sb, 0, m)))
    for qq in range(1, NQQ):
        units.append((16 * qq, proj_qk_thunks(qT, wqb, bq_sb, 0, qq)))
    for j in range(1, NOC):
        for m in range(NQQ):
            units.append((64 * j + 4 * m, proj_qk_thunks(kT, wkb, bk_sb, j, m)))
        for qq in range(NQQ):
            units.append((64 * j + 16 * qq, proj_qk_thunks(qT, wqb, bq_sb, j, qq)))
    units.sort(key=lambda u: u[0])
    work = []          # flat (deadline, engine, thunk) list
    for dl, ths in units:
        for eng, th in ths:
            work.append((dl, eng, th))
    req_by_t = [0] * (NT + 1)  # cumulative thunks due before tile t starts
    for dl, _, _ in work:
        req_by_t[min(dl, NT)] += 1
    for t in range(1, NT + 1):
        req_by_t[t] += req_by_t[t - 1]
    LOOKAHEAD = 2

    # ---- attention: one global software pipeline over all tiles ----
    def emit_qk(j, qq, kc):
        qs = slice(qq * QW, (qq + 1) * QW)
        ks = slice(kc * 128, (kc + 1) * 128)
        S_t = sps.tile([128, 2 * QW], F32, tag="S")
        nc.tensor.matmul(
            S_t[:, 0:QW], kT[0:64, j, ks], qT[0:64, j, qs],
            start=True, stop=True, tile_position=(0, 0),
        )
        nc.tensor.matmul(
            S_t[:, QW : 2 * QW], kT[64:128, j, ks], qT[64:128, j, qs],
            start=True, stop=True, tile_position=(64, 0),
        )
        return S_t

    def tail_exp(S_t):
        ex = expool.tile([128, 2 * QW], BF16, tag="ex")
        nc.scalar.activation(ex[:], S_t[:], Exp)
        return ex

    def tail_mult(ex, qq, kc):
        qs = slice(qq * QW, (qq + 1) * QW)
        pr = prpool.tile([128, 2 * QW], BF16, tag="pr")
        et_ap = ET[:, kc, qs]
        et_b = bass.AP(
            tensor=et_ap.tensor, offset=et_ap.offset,
            ap=[et_ap.ap[0], [0, 2], *et_ap.ap[1:]],
        )
        nc.vector.tensor_tensor(
            pr[:].rearrange("p (g q) -> p g q", g=2),
            ex[:].rearrange("p (g q) -> p g q", g=2),
            et_b,
            op=mybir.AluOpType.mult,
        )
        return pr

    def tail_pv(pr, j, kc, ctxA, ctxB):
        nc.tensor.matmul(
            ctxA[:], vaug[:, kc, 2 * j, 0 : D + 1], pr[:, 0:QW],
            start=(kc == 0), stop=(kc == NKC - 1),
        )
        nc.tensor.matmul(
            ctxB[:], vaug[:, kc, 2 * j + 1, 0 : D + 1], pr[:, QW : 2 * QW],
            start=(kc == 0), stop=(kc == NKC - 1),
        )

    def finish_block(j, qq, ctxA, ctxB):
        # unnormalized ctx^T (+ denominator row 64) -> SBUF bf16 -> HBM,
        # per head so the copy/DMA of head A overlaps the copy of head B
        osb = owork.tile([D + 1, 2, QW], BF16, tag="osb")
        for hh, cpsum in ((0, ctxA), (1, ctxB)):
            nc.vector.tensor_copy(osb[:, hh, :], cpsum[:])
            nc.sync.dma_start(
                out=h["out_u"].ap()[j, hh, :, qq * QW : (qq + 1) * QW],
                in_=osb[:, hh, :],
            )

    # initial projection units (everything tile 0 and the first tails need)
    for _, th in (
        proj_qk_thunks(qT, wqb, bq_sb, 0, 0)
        + proj_qk_thunks(kT, wkb, bk_sb, 0, 0)
        + proj_v_thunks(0)
        + proj_v_thunks(1)
    ):
        th()

    # Emission order: around each block boundary, interleave the last two
    # tiles of block B with the first tiles of B+1 -> every QK's S-slot
    # gating exp is always two EMISSION steps back, and ScalarE processes
    # exps back-to-back through the boundary. Tails (mult+PV) still run in
    # ORIGINAL tile order (PV accumulation + ctx-psum rotation require it).
    order = list(range(NT))
    for Bb in range(1, NOC * NQQ):
        p = 16 * Bb
        # [B:13..15, B1:0..2] -> [B1:0, B:13, B1:1, B:14, B1:2, B:15]
        order[p - 3 : p + 3] = [p, p - 3, p + 1, p - 2, p + 2, p - 1]

    wi = 0               # next work-thunk index
    emitted = {}         # original tile index -> (ex tile, emission pos)
    prs = {}             # original tile index -> pr tile (mult done, PV not)
    nm = 0               # next ORIGINAL tile index to mult
    npv = 0              # next ORIGINAL tile index to PV
    cur_ctx = None       # (ctxA, ctxB, j, qq) of the block being accumulated
    for pos in range(NT + 2):
        if pos < NT:
            t = order[pos]
            jj = t // (NQQ * NKC)
            qq = (t // NKC) % NQQ
            kc = t % NKC
            # filler BEFORE this tile's QK: cover every tile emitted in the
            # next LOOKAHEAD positions (emission order!), and drain at least
            # one spare thunk per tile to smooth the PE load. DVE-side
            # thunks go after this tile's mult (so they never delay the
            # prob-mult in the in-order DVE queue).
            tcov = max(order[pos : pos + LOOKAHEAD])
            target = max(req_by_t[min(tcov + 1, NT)], min(wi + 1, len(work)))
            dve_thunks = []
            while wi < target:
                _, eng, th = work[wi]
                if eng == "pe":
                    th()
                else:
                    dve_thunks.append(th)
                wi += 1
            # build-time check: all projection units this tile reads are
            # fully emitted (deadline pulls + rate drain guarantee it)
            assert wi >= len(work) or work[wi][0] > t, (t, wi, work[wi][0])
            S_t = emit_qk(jj, qq, kc)
            emitted[t] = (tail_exp(S_t), pos)
        else:
            dve_thunks = []
        # mult stream: ORIGINAL order, lag >= 3 emissions, >= 2 positions
        # after the tile's own emission. PV stream: 2 tiles behind mults, so
        # at block boundaries the next block's mults run on DVE BEFORE the
        # old block's ctx copies, and the PE always has queued QK work
        # between a PV and the exp it transitively feeds.
        atend = pos >= NT
        while (
            nm in emitted
            and (atend or emitted[nm][1] <= pos - 2)
            and (atend or (pos + 1) - nm > 3)
        ):
            pex, _ = emitted.pop(nm)
            prs[nm] = tail_mult(pex, (nm // NKC) % NQQ, nm % NKC)
            nm += 1
        while npv in prs and (atend or (pos + 1) - npv > 5):
            pr = prs.pop(npv)
            pj = npv // (NQQ * NKC)
            pqq = (npv // NKC) % NQQ
            pkc = npv % NKC
            if pkc == 0:
                # rotate ctx PSUM lazily, right before the first PV write
                ctxA = cps.tile([D + 1, QW], F32, tag="cA")
                ctxB = cps.tile([D + 1, QW], F32, tag="cB")
                cur_ctx = (ctxA, ctxB, pj, pqq)
            tail_pv(pr, pj, pkc, cur_ctx[0], cur_ctx[1])
            if pkc == NKC - 1:
                finish_block(cur_ctx[2], cur_ctx[3], cur_ctx[0], cur_ctx[1])
            npv += 1
        for th in dve_thunks:
            th()


def build():
    nc = bacc.Bacc("TRN2", target_bir_lowering=False, debug=False, num_devices=N_CORES)
    h = {
        "xT": nc.dram_tensor("xT", [E, S], BF16, kind="ExternalInput"),
        "wqT": nc.dram_tensor("wqT", [E, EC], BF16, kind="ExternalInput"),
        "wkT": nc.dram_tensor("wkT", [E, EC], BF16, kind="ExternalInput"),
        "wvT": nc.dram_tensor("wvT", [E, EC], BF16, kind="ExternalInput"),
        "bq": nc.dram_tensor("bq", [128, NOC], F32, kind="ExternalInput"),
        "bk": nc.dram_tensor("bk", [128, NOC], F32, kind="ExternalInput"),
        "emaskT": nc.dram_tensor("emaskT", [S, S], BF16, kind="ExternalInput"),
        "out_u": nc.dram_tensor("out_u", [NOC, 2, D + 1, S], BF16, kind="ExternalOutput"),
    }
    with tile.TileContext(nc) as tc:
        with ExitStack() as ctx:
            _emit(ctx, tc, h)
    nc.compile()
    return nc


def prep_in_maps(inputs):
    hs = np.asarray(inputs["hidden_states"], dtype=np.float32)
    am = np.asarray(inputs["attention_mask"], dtype=np.float32)
    dm = np.asarray(inputs["domain_attn_mask"], dtype=np.float32)
    Wq = np.asarray(inputs["Wq"], dtype=np.float32)
    bq = np.asarray(inputs["bq"], dtype=np.float32)
    Wk = np.asarray(inputs["Wk"], dtype=np.float32)
    bk = np.asarray(inputs["bk"], dtype=np.float32)
    Wv = np.asarray(inputs["Wv"], dtype=np.float32)
    bv = np.asarray(inputs["bv"], dtype=np.float32)

    emaskT = [
        np.exp(dm[b, 0].T + am[b, 0, 0, :, None]).astype(ml_dtypes.bfloat16)
        for b in range(B)
    ]

    in_maps = []
    for c in range(N_CORES):
        b = c // 2
        e0 = (c % 2) * EC
        sl = slice(e0, e0 + EC)
        in_maps.append(
            {
                "xT": np.ascontiguousarray(hs[b].T).astype(ml_dtypes.bfloat16),
                "wqT": (np.ascontiguousarray(Wq[sl, :].T) * 0.125).astype(
                    ml_dtypes.bfloat16
                ),
                "wkT": np.ascontiguousarray(Wk[sl, :].T).astype(ml_dtypes.bfloat16),
                "wvT": np.ascontiguousarray(Wv[sl, :].T).astype(ml_dtypes.bfloat16),
                "bq": np.ascontiguousarray((bq[sl] * 0.125).reshape(NOC, 128).T),
                "bk": np.ascontiguousarray(bk[sl].reshape(NOC, 128).T),
                "emaskT": emaskT[b],
            }
        )
    return in_maps


def finalize_core(u, bv_c):
    """u: [NOC, 2, D+1, S] unnormalized ctx^T (v WITHOUT bias) -> [S, EC]
    normalized ctx. The v bias commutes with the softmax average, so it is
    added here: softmax @ (v + bv) = softmax @ v + bv."""
    u = np.asarray(u, dtype=np.float32)
    ctxn = u[:, :, 0:D, :] / u[:, :, D : D + 1, :]
    return np.ascontiguousarray(ctxn.transpose(3, 0, 1, 2).reshape(S, EC)) + bv_c


_cached_nc = None


def run(inputs, trace=False):
    global _cached_nc
    if _cached_nc is None:
        _cached_nc = build()
    in_maps = prep_in_maps(inputs)
    res = run_bass_kernel_spmd(
        _cached_nc, in_maps, core_ids=list(range(N_CORES)), trace=trace
    )
    bv = np.asarray(inputs["bv"], dtype=np.float32)
    out = np.empty((B, S, E), dtype=np.float32)
    for c in range(N_CORES):
        b = c // 2
        e0 = (c % 2) * EC
        out[b, :, e0 : e0 + EC] = finalize_core(
            res.results[c]["out_u"], bv[e0 : e0 + EC]
        )
    return out, res


def kernel(**inputs) -> np.ndarray:
    return run(inputs)[0]


# revision 51
# speedup vs baseline: 1.2135x; 1.2135x over previous
"""BERT self-attention (B=4, S=2048, E=768, H=12) on 8 TRN2 NeuronCores.

Sharding: (batch, head-half) — core c handles batch c//2, heads 6*(c%2)..+6.
Each core is fully independent (no collectives).

Host-side prep (in kernel()): per-core shard slicing plus layout/precision
prep — hidden/W transposed to put the contraction dim on partitions, Wq/bq
pre-scaled by 1/sqrt(D), attention_mask folded into domain mask and the
combined mask EXPONENTIATED on the host (E_T = exp(maskT) ships as bf16, so
ScalarE never touches the masks), matmul operands fed as bf16.

Device-side structure (per core):
  - projections (bf16): qT,kT in [o,m] layout; v in [m,o] layout augmented
    with a ones column per head (softmax denominators via the PV matmul).
  - scores^T[k,q] = kT.T @ qT, two heads row-packed per PE pass (d=64 each)
    into one f32 PSUM tile [128, 1024].
  - one ACT pass per k-chunk: exp(scores) PSUM -> SBUF bf16 (the ScalarE
    bottleneck, ~1.0 us per 128x1024 tile).
  - host-precomputed E_T = exp(maskT) multiplied in at bf16 2x on DVE:
    prod = exp_s * E_T.
  - PV: ctx_u^T[65,q] = v_aug.T @ prod accumulated over 16 k-chunks in
    PSUM; row 64 is the softmax denominator.
  - ctx_u^T is copied f32 PSUM->SBUF and DMA'd out UNNORMALIZED; the host
    divides rows 0..63 by row 64 and transposes to [q, e]. This removes
    the per-block PE transposes + DVE normalize from the device entirely.

Pipelining: one global software pipeline over all 192 (q-block, head-pair,
k-chunk) tiles — the next tile's QK matmuls are always emitted before the
previous tile's exp/mult/PV tail, so neither PE nor ScalarE stalls at block
boundaries. Projections are interleaved as filler into the PE slack of the
ACT-bound k-loop with just-in-time deadlines.

Measured on 8 axon TRN2 cores: see test.py output.
"""

import sys

if "/opt/trn_rl_repo" not in sys.path:
    sys.path.insert(0, "/opt/trn_rl_repo")

from contextlib import ExitStack

import ml_dtypes
import numpy as np

import concourse.bass as bass
import concourse.tile as tile
from concourse import bacc, mybir
from concourse.bass_utils import run_bass_kernel_spmd

B, S, E, H = 4, 2048, 768, 12
D = 64
N_CORES = 8
HPC = 6            # heads per core
EC = HPC * D       # 384 embedding cols per core
NIC = E // 128     # 6 contraction chunks
NOC = EC // 128    # 3 output chunks (= head pairs)
NKC = S // 128     # 16 k chunks
QW = 512           # q tile width
NQQ = S // QW      # 4 q chunks
NT = NOC * NQQ * NKC  # 192 tiles total

F32 = mybir.dt.float32
BF16 = mybir.dt.bfloat16
FP8 = mybir.dt.float8e4
DR = mybir.MatmulPerfMode.DoubleRow
Exp = mybir.ActivationFunctionType.Exp


def _emit(ctx: ExitStack, tc: tile.TileContext, h):
    nc = tc.nc

    persist = ctx.enter_context(tc.tile_pool(name="persist", bufs=1))
    consts = ctx.enter_context(tc.tile_pool(name="consts", bufs=1))

    bq_sb = consts.tile([128, NOC], F32)
    bk_sb = consts.tile([128, NOC], F32)
    ones1 = consts.tile([1, 128], BF16)
    scratch1 = consts.tile([1, 1], BF16)

    # ---- persistent activations ----
    qT = persist.tile([128, NOC, S], BF16)        # [o%128, o-chunk, m]
    kT = persist.tile([128, NOC, S], BF16)
    vaug = persist.tile([128, NKC, HPC, D + 4], BF16)  # [m%128, m-chunk, head, d|one]
    ET = persist.tile([128, NKC, S], BF16)        # host exp(maskT), [k%128, k-chunk, q]

    # stage A/B inputs stay resident the whole run (projections interleave
    # into the attention loop)
    sab = ctx.enter_context(tc.tile_pool(name="stageAB", bufs=1))
    xTb = sab.tile([128, NIC, S], BF16)
    wqb = sab.tile([128, NIC, EC], BF16)
    wkb = sab.tile([128, NIC, EC], BF16)
    wvb = sab.tile([128, NIC, EC], BF16)

    # ---- input DMAs: FIRST emissions, spread across three trigger queues
    # so the ramp-critical set (wq/wk + xT mq0) is all in flight within a
    # few triggers of kernel start. ScalarE is idle until the first exp
    # (~17us), so it can serve as a trigger queue for the wk loads.
    def load_x(mq, c):
        qs = slice(mq * QW, (mq + 1) * QW)
        nc.sync.dma_start(
            out=xTb[:, 2 * c : 2 * c + 2, qs],
            in_=h["xT"].ap()[c * 256 : (c + 1) * 256, qs].rearrange(
                "(a p) q -> p a q", p=128
            ),
        )

    def load_w(eng, name, wtb, c):
        eng.dma_start(
            out=wtb[:, 2 * c : 2 * c + 2, :],
            in_=h[name].ap()[c * 256 : (c + 1) * 256, :].rearrange(
                "(a p) o -> p a o", p=128
            ),
        )

    def load_et(eng, kc, qq):
        qs = slice(qq * QW, (qq + 1) * QW)
        eng.dma_start(
            out=ET[:, kc, qs], in_=h["emaskT"].ap()[kc * 128 : (kc + 1) * 128, qs]
        )

    # E_T streams by q-quarter in block consumption order: block (j0,qq0)
    # only reads q-columns 0:512 of every k-chunk, so the first 2.1MB of
    # mask unblocks the whole first block instead of the full 8.4MB.
    for c in range(NIC // 2):
        load_x(0, c)
        load_w(nc.scalar, "wkT", wkb, c)
        load_w(nc.gpsimd, "wqT", wqb, c)
    for kc in range(4):
        load_et(nc.sync, kc, 0)
    for c in range(NIC // 2):
        load_x(1, c)
        load_w(nc.gpsimd, "wvT", wvb, c)
    nc.gpsimd.dma_start(out=bq_sb[:], in_=h["bq"].ap())
    nc.gpsimd.dma_start(out=bk_sb[:], in_=h["bk"].ap())
    for kc in range(4, 10):
        load_et(nc.sync, kc, 0)
    for c in range(NIC // 2):
        load_x(2, c)
    for kc in range(10, NKC):
        load_et(nc.sync, kc, 0)
    for c in range(NIC // 2):
        load_x(3, c)
    for qq in range(1, NQQ):
        for kc in range(NKC):
            load_et(nc.gpsimd, kc, qq)

    # ---- constants (after the DMA triggers so they don't delay them) ----
    nc.vector.memset(ones1[:], 1.0)
    nc.vector.memset(vaug[:, :, :, D : D + 1], 1.0)
    # dummy exp at t~0: pulls the ACT exp-table load off the critical path
    nc.scalar.activation(scratch1[:], ones1[0:1, 0:1], Exp)

    # ---- working pools ----
    sps = ctx.enter_context(tc.tile_pool(name="s_psum", bufs=2, space="PSUM"))
    cps = ctx.enter_context(tc.tile_pool(name="ctx_psum", bufs=1, space="PSUM"))
    pps = ctx.enter_context(tc.tile_pool(name="proj_psum", bufs=2, space="PSUM"))
    expool = ctx.enter_context(tc.tile_pool(name="expool", bufs=10))
    prpool = ctx.enter_context(tc.tile_pool(name="prpool", bufs=6))
    owork = ctx.enter_context(tc.tile_pool(name="owork", bufs=3))

    # ---- projection units, split into single-matmul thunks so they can be
    # interleaved finely into the PE slack of the ACT-bound k-loop ----
    def proj_qk_thunks(dst, wtb, bias, oc, mq):
        st = {}

        def mm(ic):
            def f():
                if ic == 0:
                    st["ps"] = pps.tile([128, QW], F32, tag="pp", name="projps")
                nc.tensor.matmul(
                    st["ps"][:],
                    wtb[:, ic, oc * 128 : (oc + 1) * 128],
                    xTb[:, ic, mq * QW : (mq + 1) * QW],
                    start=(ic == 0),
                    stop=(ic == NIC - 1),
                )
            return f

        def fin():
            nc.vector.tensor_scalar_add(
                dst[:, oc, mq * QW : (mq + 1) * QW], st["ps"][:], bias[:, oc : oc + 1]
            )

        return [("pe", mm(ic)) for ic in range(NIC)] + [("dve", fin)]

    def proj_v_thunks(mc):
        st = {}

        def mm(ic):
            def f():
                if ic == 0:
                    st["ps"] = pps.tile([128, QW], F32, tag="pp", name="projps")
                nc.tensor.matmul(
                    st["ps"][:, 0:EC],
                    xTb[:, ic, mc * 128 : (mc + 1) * 128],
                    wvb[:, ic, :],
                    start=(ic == 0),
                    stop=(ic == NIC - 1),
                )
            return f

        def fin():
            nc.vector.tensor_copy(
                vaug[:, mc, :, 0:D], st["ps"][:, 0:EC].rearrange("p (h d) -> p h d", h=HPC)
            )

        return [("pe", mm(ic)) for ic in range(NIC)] + [("dve", fin)]

    # (deadline_tile, thunks) in consumption order:
    #   v(mc) consumed at tile mc+1; kT(0,m) at 4m; qT(0,qq) at 16qq;
    #   kT(j,m) at 64j (first block of j); qT(j,qq) at 64j+16qq.
    units = []
    for mc in range(2, NKC):
        units.append((mc + 1, proj_v_thunks(mc)))
    for m in range(1, NQQ):
        units.append((4 * m, proj_qk_thunks(kT, wkb, bk_sb, 0, m)))
    for qq in range(1, NQQ):
        units.append((16 * qq, proj_qk_thunks(qT, wqb, bq_sb, 0, qq)))
    for j in range(1, NOC):
        for m in range(NQQ):
            units.append((64 * j + 4 * m, proj_qk_thunks(kT, wkb, bk_sb, j, m)))
        for qq in range(NQQ):
            units.append((64 * j + 16 * qq, proj_qk_thunks(qT, wqb, bq_sb, j, qq)))
    units.sort(key=lambda u: u[0])
    work = []          # flat (deadline, engine, thunk) list
    for dl, ths in units:
        for eng, th in ths:
            work.append((dl, eng, th))
    req_by_t = [0] * (NT + 1)  # cumulative thunks due before tile t starts
    for dl, _, _ in work:
        req_by_t[min(dl, NT)] += 1
    for t in range(1, NT + 1):
        req_by_t[t] += req_by_t[t - 1]
    LOOKAHEAD = 2

    # ---- attention: one global software pipeline over all tiles ----
    def emit_qk(j, qq, kc):
        qs = slice(qq * QW, (qq + 1) * QW)
        ks = slice(kc * 128, (kc + 1) * 128)
        S_t = sps.tile([128, 2 * QW], F32, tag="S")
        nc.tensor.matmul(
            S_t[:, 0:QW], kT[0:64, j, ks], qT[0:64, j, qs],
            start=True, stop=True, tile_position=(0, 0),
        )
        nc.tensor.matmul(
            S_t[:, QW : 2 * QW], kT[64:128, j, ks], qT[64:128, j, qs],
            start=True, stop=True, tile_position=(64, 0),
        )
        return S_t

    def tail_exp(S_t):
        ex = expool.tile([128, 2 * QW], BF16, tag="ex")
        nc.scalar.activation(ex[:], S_t[:], Exp)
        return ex

    def tail_mult(ex, qq, kc):
        qs = slice(qq * QW, (qq + 1) * QW)
        pr = prpool.tile([128, 2 * QW], BF16, tag="pr")
        et_ap = ET[:, kc, qs]
        et_b = bass.AP(
            tensor=et_ap.tensor, offset=et_ap.offset,
            ap=[et_ap.ap[0], [0, 2], *et_ap.ap[1:]],
        )
        nc.vector.tensor_tensor(
            pr[:].rearrange("p (g q) -> p g q", g=2),
            ex[:].rearrange("p (g q) -> p g q", g=2),
            et_b,
            op=mybir.AluOpType.mult,
        )
        return pr

    def tail_pv(pr, j, kc, ctxA, ctxB):
        nc.tensor.matmul(
            ctxA[:], vaug[:, kc, 2 * j, 0 : D + 1], pr[:, 0:QW],
            start=(kc == 0), stop=(kc == NKC - 1),
        )
        nc.tensor.matmul(
            ctxB[:], vaug[:, kc, 2 * j + 1, 0 : D + 1], pr[:, QW : 2 * QW],
            start=(kc == 0), stop=(kc == NKC - 1),
        )

    def finish_block(j, qq, ctxA, ctxB):
        # unnormalized ctx^T (+ denominator row 64) -> SBUF bf16 -> HBM,
        # per head so the copy/DMA of head A overlaps the copy of head B
        osb = owork.tile([D + 1, 2, QW], BF16, tag="osb")
        for hh, cpsum in ((0, ctxA), (1, ctxB)):
            nc.vector.tensor_copy(osb[:, hh, :], cpsum[:])
            nc.sync.dma_start(
                out=h["out_u"].ap()[j, hh, :, qq * QW : (qq + 1) * QW],
                in_=osb[:, hh, :],
            )

    # initial projection units (everything tile 0 and the first tails need)
    for _, th in (
        proj_qk_thunks(qT, wqb, bq_sb, 0, 0)
        + proj_qk_thunks(kT, wkb, bk_sb, 0, 0)
        + proj_v_thunks(0)
        + proj_v_thunks(1)
    ):
        th()

    # Emission order: around each block boundary, interleave the last two
    # tiles of block B with the first tiles of B+1 -> every QK's S-slot
    # gating exp is always two EMISSION steps back, and ScalarE processes
    # exps back-to-back through the boundary. Tails (mult+PV) still run in
    # ORIGINAL tile order (PV accumulation + ctx-psum rotation require it).
    order = list(range(NT))
    for Bb in range(1, NOC * NQQ):
        p = 16 * Bb
        # [B:13..15, B1:0..2] -> [B1:0, B:13, B1:1, B:14, B1:2, B:15]
        order[p - 3 : p + 3] = [p, p - 3, p + 1, p - 2, p + 2, p - 1]

    wi = 0               # next work-thunk index
    emitted = {}         # original tile index -> (ex tile, emission pos)
    prs = {}             # original tile index -> pr tile (mult done, PV not)
    nm = 0               # next ORIGINAL tile index to mult
    npv = 0              # next ORIGINAL tile index to PV
    cur_ctx = None       # (ctxA, ctxB, j, qq) of the block being accumulated
    for pos in range(NT + 2):
        if pos < NT:
            t = order[pos]
            jj = t // (NQQ * NKC)
            qq = (t // NKC) % NQQ
            kc = t % NKC
            # filler BEFORE this tile's QK: cover every tile emitted in the
            # next LOOKAHEAD positions (emission order!), and drain at least
            # one spare thunk per tile to smooth the PE load. DVE-side
            # thunks go after this tile's mult (so they never delay the
            # prob-mult in the in-order DVE queue).
            tcov = max(order[pos : pos + LOOKAHEAD])
            target = max(req_by_t[min(tcov + 1, NT)], min(wi + 1, len(work)))
            dve_thunks = []
            while wi < target:
                _, eng, th = work[wi]
                if eng == "pe":
                    th()
                else:
                    dve_thunks.append(th)
                wi += 1
            # build-time check: all projection units this tile reads are
            # fully emitted (deadline pulls + rate drain guarantee it)
            assert wi >= len(work) or work[wi][0] > t, (t, wi, work[wi][0])
            S_t = emit_qk(jj, qq, kc)
            emitted[t] = (tail_exp(S_t), pos)
        else:
            dve_thunks = []
        # mult stream: ORIGINAL order, lag >= 3 emissions, >= 2 positions
        # after the tile's own emission. PV stream: 2 tiles behind mults, so
        # at block boundaries the next block's mults run on DVE BEFORE the
        # old block's ctx copies, and the PE always has queued QK work
        # between a PV and the exp it transitively feeds.
        atend = pos >= NT
        while (
            nm in emitted
            and (atend or emitted[nm][1] <= pos - 2)
            and (atend or (pos + 1) - nm > 3)
        ):
            pex, _ = emitted.pop(nm)
            prs[nm] = tail_mult(pex, (nm // NKC) % NQQ, nm % NKC)
            nm += 1
        while npv in prs and (atend or (pos + 1) - npv > 5):
            pr = prs.pop(npv)
            pj = npv // (NQQ * NKC)
            pqq = (npv // NKC) % NQQ
            pkc = npv % NKC
            if pkc == 0:
                # rotate ctx PSUM lazily, right before the first PV write
                ctxA = cps.tile([D + 1, QW], F32, tag="cA")
                ctxB = cps.tile([D + 1, QW], F32, tag="cB")
                cur_ctx = (ctxA, ctxB, pj, pqq)
            tail_pv(pr, pj, pkc, cur_ctx[0], cur_ctx[1])
            if pkc == NKC - 1:
                finish_block(cur_ctx[2], cur_ctx[3], cur_ctx[0], cur_ctx[1])
            npv += 1
        for th in dve_thunks:
            th()


def build():
    nc = bacc.Bacc("TRN2", target_bir_lowering=False, debug=False, num_devices=N_CORES)
    h = {
        "xT": nc.dram_tensor("xT", [E, S], BF16, kind="ExternalInput"),
        "wqT": nc.dram_tensor("wqT", [E, EC], BF16, kind="ExternalInput"),
        "wkT": nc.dram_tensor("wkT", [E, EC], BF16, kind="ExternalInput"),
        "wvT": nc.dram_tensor("wvT", [E, EC], BF16, kind="ExternalInput"),
        "bq": nc.dram_tensor("bq", [128, NOC], F32, kind="ExternalInput"),
        "bk": nc.dram_tensor("bk", [128, NOC], F32, kind="ExternalInput"),
        "emaskT": nc.dram_tensor("emaskT", [S, S], BF16, kind="ExternalInput"),
        "out_u": nc.dram_tensor("out_u", [NOC, 2, D + 1, S], BF16, kind="ExternalOutput"),
    }
    with tile.TileContext(nc) as tc:
        with ExitStack() as ctx:
            _emit(ctx, tc, h)
    nc.compile()
    return nc


def prep_in_maps(inputs):
    hs = np.asarray(inputs["hidden_states"], dtype=np.float32)
    am = np.asarray(inputs["attention_mask"], dtype=np.float32)
    dm = np.asarray(inputs["domain_attn_mask"], dtype=np.float32)
    Wq = np.asarray(inputs["Wq"], dtype=np.float32)
    bq = np.asarray(inputs["bq"], dtype=np.float32)
    Wk = np.asarray(inputs["Wk"], dtype=np.float32)
    bk = np.asarray(inputs["bk"], dtype=np.float32)
    Wv = np.asarray(inputs["Wv"], dtype=np.float32)
    bv = np.asarray(inputs["bv"], dtype=np.float32)

    emaskT = [
        np.exp(dm[b, 0].T + am[b, 0, 0, :, None]).astype(ml_dtypes.bfloat16)
        for b in range(B)
    ]

    in_maps = []
    for c in range(N_CORES):
        b = c // 2
        e0 = (c % 2) * EC
        sl = slice(e0, e0 + EC)
        in_maps.append(
            {
                "xT": np.ascontiguousarray(hs[b].T).astype(ml_dtypes.bfloat16),
                "wqT": (np.ascontiguousarray(Wq[sl, :].T) * 0.125).astype(
                    ml_dtypes.bfloat16
                ),
                "wkT": np.ascontiguousarray(Wk[sl, :].T).astype(ml_dtypes.bfloat16),
                "wvT": np.ascontiguousarray(Wv[sl, :].T).astype(ml_dtypes.bfloat16),
                "bq": np.ascontiguousarray((bq[sl] * 0.125).reshape(NOC, 128).T),
                "bk": np.ascontiguousarray(bk[sl].reshape(NOC, 128).T),
                "emaskT": emaskT[b],
            }
        )
    return in_maps


def finalize_core(u, bv_c):
    """u: [NOC, 2, D+1, S] unnormalized ctx^T (v WITHOUT bias) -> [S, EC]
    normalized ctx. The v bias commutes with the softmax average, so it is
    added here: softmax @ (v + bv) = softmax @ v + bv."""
    u = np.asarray(u, dtype=np.float32)
    ctxn = u[:, :, 0:D, :] / u[:, :, D : D + 1, :]
    return np.ascontiguousarray(ctxn.transpose(3, 0, 1, 2).reshape(S, EC)) + bv_c


_cached_nc = None


def run(inputs, trace=False):
    global _cached_nc
    if _cached_nc is None:
        _cached_nc = build()
    in_maps = prep_in_maps(inputs)
    res = run_bass_kernel_spmd(
        _cached_nc, in_maps, core_ids=list(range(N_CORES)), trace=trace
    )
    bv = np.asarray(inputs["bv"], dtype=np.float32)
    out = np.empty((B, S, E), dtype=np.float32)
    for c in range(N_CORES):
        b = c // 2
        e0 = (c % 2) * EC
        out[b, :, e0 : e0 + EC] = finalize_core(
            res.results[c]["out_u"], bv[e0 : e0 + EC]
        )
    return out, res


def kernel(**inputs) -> np.ndarray:
    return run(inputs)[0]
